# revision 25
# baseline (speedup 1.0000x reference)
"""Trainium2 Bass kernel for a minimal transformer block (B=2, T=2048, C=1024,
H=16, Dh=64, F=4096), sharded over 8 NeuronCores.

Sharding: data-parallel over batch (2 groups of 4 cores) x sequence-parallel
over tokens within each batch (512 query tokens per core). K/V are computed
only for each core's own 512 tokens and exchanged within the 4-core group via
one fp8 AllGather (1MB per core), eliminating the 4x-redundant full-sequence
K/V recompute of the data-parallel-only layout.

Precision: QKV / O projections and the PV attention matmul run in fp8-e4m3
with DoubleRow packing (K=256 per pass, 2x PE throughput); attention scores
and the FFN stay bf16 (fp8 FFN does not fit the accuracy budget). LayerNorm
is folded into the projections (gamma into the weights, beta+biases into
per-feature constants, the mean term as a rank-1 correction streamed from an
on-chip -mu row); the fp8 weight descale rides the LN scale row for free.

Everything on-chip is feature-major ([features, tokens]); the host
pre-transposes, pair-packs (for DoubleRow) and quantizes the weights.
"""

import sys

if "/opt/trn_rl_repo" not in sys.path:
    sys.path.insert(0, "/opt/trn_rl_repo")

import numpy as np

D_MODEL = 1024
N_HEAD = 16
HEAD_DIM = 64
D_FF = 4096
B = 2
T = 2048
N_CORES = 8
GROUPS = 4          # cores per batch group
TQ = T // GROUPS    # own tokens per core = 512
P = 128
NCC = D_MODEL // P  # 8 feature chunks
NPP = NCC // 2      # 4 feature pair-chunks (DoubleRow)
NKC = T // P        # 16 key chunks of 128
NKP = NKC // 2      # 8 key chunk pairs
NFC = D_FF // P     # 32 f-chunks

KQ_SCALE = 2048.0   # fp8 scale for qkv weights
KO_SCALE = 2048.0   # fp8 scale for o weights

# bias-table column layout ([128, 64] f32)
QB, B1, B2, DSC = 0, 8, 40, 48

_cache = {}


def _build():
    import concourse.bass as bass
    import concourse.tile as tile
    from concourse import bacc, mybir

    f32 = mybir.dt.float32
    bf16 = mybir.dt.bfloat16
    f8 = mybir.dt.float8e4
    AF = mybir.ActivationFunctionType
    OP = mybir.AluOpType
    DR = mybir.MatmulPerfMode.DoubleRow

    nc = bacc.Bacc("TRN2", target_bir_lowering=False, debug=False,
                   num_devices=N_CORES)

    x8_d = nc.dram_tensor("x8p", [512, 2 * TQ], f8, kind="ExternalInput").ap()
    xo_d = nc.dram_tensor("x_own", [D_MODEL, TQ], f32,
                          kind="ExternalInput").ap()
    qkv8_d = nc.dram_tensor("qkv8p", [512, 2 * 3 * D_MODEL], f8,
                            kind="ExternalInput").ap()
    ow8_d = nc.dram_tensor("ow8p", [512, 2 * D_MODEL], f8,
                           kind="ExternalInput").ap()
    wbar_d = nc.dram_tensor("wbar", [1, 3 * D_MODEL], bf16,
                            kind="ExternalInput").ap()
    wbar1_d = nc.dram_tensor("wbar1", [1, D_FF], bf16,
                             kind="ExternalInput").ap()
    w1_d = nc.dram_tensor("w1T", [D_MODEL, D_FF], bf16,
                          kind="ExternalInput").ap()
    w2_d = nc.dram_tensor("w2T", [D_FF, D_MODEL], bf16,
                          kind="ExternalInput").ap()
    bias_d = nc.dram_tensor("biases", [P, 64], f32, kind="ExternalInput").ap()
    out_d = nc.dram_tensor("out", [D_MODEL, TQ], f32,
                           kind="ExternalOutput").ap()
    dbg = None
    if _cache.get("debug"):
        dbg = {
            "dbg_k0": nc.dram_tensor("dbg_k0", [P, T], f8,
                                     kind="ExternalOutput").ap(),
            "dbg_v0": nc.dram_tensor("dbg_v0", [P, 2 * N_HEAD * 80], f8,
                                     kind="ExternalOutput").ap(),
            "dbg_q0": nc.dram_tensor("dbg_q0", [P, TQ], bf16,
                                     kind="ExternalOutput").ap(),
            "dbg_es0": nc.dram_tensor("dbg_es0", [P, 2 * TQ], f8,
                                      kind="ExternalOutput").ap(),
            "dbg_x20": nc.dram_tensor("dbg_x20", [P, TQ], f32,
                                      kind="ExternalOutput").ap(),
            "dbg_sbb": nc.dram_tensor("dbg_sbb", [P, TQ], bf16,
                                      kind="ExternalOutput").ap(),
            "dbg_nmu": nc.dram_tensor("dbg_nmu", [1, TQ], bf16,
                                      kind="ExternalOutput").ap(),
            "dbg_kvall": nc.dram_tensor("dbg_kvall",
                                        [GROUPS * 2 * D_MODEL, TQ], f8,
                                        kind="ExternalOutput").ap(),
            "dbg_attn0": nc.dram_tensor("dbg_attn0", [P, 2 * TQ], f8,
                                        kind="ExternalOutput").ap(),
            "dbg_z": nc.dram_tensor("dbg_z", [1, TQ], f32,
                                    kind="ExternalOutput").ap(),
            "dbg_rcp": nc.dram_tensor("dbg_rcp", [1, TQ], bf16,
                                      kind="ExternalOutput").ap(),
            "dbg_rbs": nc.dram_tensor("dbg_rbs", [64, TQ], f32,
                                      kind="ExternalOutput").ap(),
        }

    with tile.TileContext(nc) as tc:
        _body(tc, bass, mybir, f32, bf16, f8, AF, OP, DR,
              x8_d, xo_d, qkv8_d, ow8_d, wbar_d, wbar1_d, w1_d, w2_d,
              bias_d, out_d, dbg)

    nc.compile()
    return nc


def _body(tc, bass, mybir, f32, bf16, f8, AF, OP, DR,
          x8_d, xo_d, qkv8_d, ow8_d, wbar_d, wbar1_d, w1_d, w2_d,
          bias_d, out_d, dbg=None):
    nc = tc.nc
    from contextlib import ExitStack

    inv_n = 1.0 / D_MODEL
    rs = 1.0 / np.sqrt(HEAD_DIM)

    ctx = ExitStack()
    with ctx:
        # ---------------- persistent arenas ----------------
        const_pool = ctx.enter_context(tc.tile_pool(name="const", bufs=1))
        ka = ctx.enter_context(tc.tile_pool(name="ka", bufs=1))
        va = ctx.enter_context(tc.tile_pool(name="va", bufs=1))
        esa = ctx.enter_context(tc.tile_pool(name="esa", bufs=1))
        attn8p = ctx.enter_context(tc.tile_pool(name="attn8", bufs=1))
        x2a = ctx.enter_context(tc.tile_pool(name="x2a", bufs=1))
        xbca = ctx.enter_context(tc.tile_pool(name="xbca", bufs=1))
        sln = ctx.enter_context(tc.tile_pool(name="sln", bufs=1))
        ow8a = ctx.enter_context(tc.tile_pool(name="ow8", bufs=1))
        qa = ctx.enter_context(tc.tile_pool(name="qa", bufs=1))
        xoa = ctx.enter_context(tc.tile_pool(name="xoa", bufs=1))
        dram = ctx.enter_context(tc.tile_pool(name="kvdram", bufs=1,
                                              space="DRAM"))

        # ---- constants ----
        bias_sb = const_pool.tile([P, 64], f32, tag="bias", name="bias")
        nc.sync.dma_start(bias_sb[:], bias_d[:])
        wbar_sb = const_pool.tile([1, 3 * D_MODEL], bf16, tag="wbar",
                                  name="wbar")
        nc.sync.dma_start(wbar_sb[:], wbar_d[:])
        wbar1_sb = const_pool.tile([1, D_FF], bf16, tag="wbar1", name="wbar1")
        nc.sync.dma_start(wbar1_sb[:], wbar1_d[:])
        ones_bf = const_pool.tile([P, 1], bf16, tag="ones_bf", name="ones_bf")
        nc.vector.memset(ones_bf[:], 1.0)
        ones_row = const_pool.tile([1, P], bf16, tag="ones_row",
                                   name="ones_row")
        nc.vector.memset(ones_row[:], 1.0)
        ones_f32 = const_pool.tile([1, P], f32, tag="ones_f32",
                                   name="ones_f32")
        nc.vector.memset(ones_f32[:], 1.0)

        def bcol(base, i=0):
            return bias_sb[:, base + i:base + i + 1]

        # ---- persistent data tiles ----
        k_sb = [ka.tile([P, T], f8, tag=f"k{i}", name=f"k{i}")
                for i in range(NCC)]
        # 80-wide per-head V blocks: 64 values + ones col (Z) + 15 zero
        # pad cols, so the DoubleRow stationary AP stays 16-byte aligned.
        v_sb = [va.tile([P, 2 * N_HEAD * 80], f8, tag=f"v{i}", name=f"v{i}")
                for i in range(NKP)]
        v4 = [t.rearrange("p (two h c) -> p two h c", two=2, c=80)
              for t in v_sb]
        es_t = [esa.tile([P, 2 * TQ], f8, tag=f"es{i}", name=f"es{i}")
                for i in range(2 * NKP)]
        es3 = [t.rearrange("p (two n) -> p two n", two=2) for t in es_t]
        attn8 = [attn8p.tile([P, 2 * TQ], f8, tag=f"a8{i}", name=f"a8_{i}")
                 for i in range(NPP)]
        attn83 = [t.rearrange("p (two n) -> p two n", two=2) for t in attn8]
        x2 = [x2a.tile([P, TQ], f32, tag=f"x2{i}", name=f"x2_{i}")
              for i in range(NCC)]
        xbc = [xbca.tile([P, TQ], bf16, tag=f"xb{i}", name=f"xbc{i}")
               for i in range(NCC)]
        q_sb = [qa.tile([P, TQ], bf16, tag=f"q{i}", name=f"q{i}")
                for i in range(NCC)]
        xo = [xoa.tile([P, TQ], f32, tag=f"xo{ci}", name=f"xo{ci}")
              for ci in range(NCC)]

        for pp in range(NKP):
            nc.vector.memset(v4[pp][:, :, :, 64:80], 0.0)
            nc.vector.memset(v4[pp][:, :, :, 64:65], 1.0)

        # LN rows
        nmu_row = sln.tile([1, TQ], bf16, tag="nmu", name="nmu_row")
        s_colf = sln.tile([P, GROUPS], f32, tag="scol", name="s_colf")
        sbb = sln.tile([P, TQ], bf16, tag="sbb", name="sbb")
        nmu2_row = sln.tile([1, TQ], bf16, tag="nmu2", name="nmu2_row")
        s2bb = sln.tile([P, TQ], bf16, tag="s2bb", name="s2_bb")

        # DRAM bounce buffers for the fp8 K/V AllGather:
        # rows 0..1023   : K feature-major [1024 feats, 512 own tokens]
        # rows 1024..2047: V token-major, token t -> rows 1024+2t, 1024+2t+1
        kv_own = dram.tile([2 * D_MODEL, TQ], f8, name="kv_own")
        kv_all = dram.tile([GROUPS * 2 * D_MODEL, TQ], f8,
                           name="kv_all")

        # mid-life arenas (close after the Q projection; their SBUF is
        # then reused by the FFN pools)
        mid = ExitStack()
        x8a = mid.enter_context(tc.tile_pool(name="x8a", bufs=1))
        qkvw8 = mid.enter_context(tc.tile_pool(name="qkvw8", bufs=1))

        x8 = []     # 4 pair tiles [128, 2x512] fp8 (own tokens)
        for g in range(NPP):
            xt = x8a.tile([P, 2 * TQ], f8, tag=f"x8{g}", name=f"x8_{g}")
            nc.sync.dma_start(xt[:], x8_d[g * P:(g + 1) * P, :])
            x8.append(xt)
        x83 = [t.rearrange("p (two n) -> p two n", two=2) for t in x8]

        qkv8 = []   # 4 pair tiles [128, 2x3072] fp8
        for g in range(NPP):
            wt = qkvw8.tile([P, 2 * 3 * D_MODEL], f8, tag=f"qw{g}",
                            name=f"qkv8_{g}")
            nc.sync.dma_start(wt[:], qkv8_d[g * P:(g + 1) * P, :])
            qkv8.append(wt)
        for ci in range(NCC):
            nc.sync.dma_start(xo[ci][:], xo_d[ci * P:(ci + 1) * P, :])
        qkv83 = [t.rearrange("p (two n) -> p two n", two=2) for t in qkv8]

        # ---------------- LN1 stats + scale rows ----------------
        with tc.tile_pool(name="xsq", bufs=2) as xsq_pool, \
             tc.tile_pool(name="ln1ps", bufs=1, space="PSUM") as lnps, \
             tc.tile_pool(name="ln1bc", bufs=2, space="PSUM") as lnbc, \
             tc.tile_pool(name="ln1t", bufs=1) as lnt:

            st = lnps.tile([33, TQ], f32, tag="st", name="st")
            for g in range(NPP):
                for i in range(2):
                    nc.tensor.matmul(st[0:1, :], ones_bf[:], x83[g][:, i, :],
                                     start=(g == 0 and i == 0),
                                     stop=(g == NPP - 1 and i == 1))
            for g in range(NPP):
                xsq = xsq_pool.tile([P, 2 * TQ], bf16, tag="xsq", name="xsq")
                nc.vector.tensor_mul(xsq[:], x8[g][:], x8[g][:])
                for i in range(2):
                    nc.tensor.matmul(st[32:33, :], ones_bf[:],
                                     xsq[:, i * TQ:(i + 1) * TQ],
                                     start=(g == 0 and i == 0),
                                     stop=(g == NPP - 1 and i == 1))

            mu_f = lnt.tile([1, TQ], f32, tag="mu", name="mu_f")
            nc.scalar.mul(mu_f[:], st[0:1, :], inv_n)
            musq = lnt.tile([1, TQ], f32, tag="musq", name="musq")
            nc.scalar.square(musq[:], mu_f[:])
            vpe = lnt.tile([1, TQ], f32, tag="vpe", name="vpe")
            nc.vector.tensor_scalar(vpe[:], st[32:33, :], inv_n, 1e-5,
                                    OP.mult, OP.add)
            nc.vector.tensor_sub(vpe[:], vpe[:], musq[:])
            rvar = lnt.tile([1, TQ], f32, tag="rvar", name="rvar")
            nc.vector.reciprocal_approx_fast(rvar[:], vpe[:])
            s_row = lnt.tile([1, TQ], bf16, tag="srow", name="s_row")
            with nc.allow_low_precision(reason="bf16 LN rows"):
                # s = rsqrt(var+eps) / KQ  (fp8 weight descale folded in)
                nc.scalar.activation(s_row[:], rvar[:], AF.Sqrt,
                                     scale=1.0 / (KQ_SCALE * KQ_SCALE))
                nc.scalar.mul(nmu_row[:], mu_f[:], -1.0)

            sb_ps = lnbc.tile([P, TQ], f32, tag="sb", name="sb_ps")
            nc.tensor.matmul(sb_ps[:], ones_row[:], s_row[:])
            nc.scalar.copy(sbb[:], sb_ps[:])
            scol_ps = lnbc.tile([P, GROUPS], f32, tag="sc", name="scol_ps")
            for k in range(GROUPS):
                nc.tensor.matmul(scol_ps[:, k:k + 1],
                                 s_row[:, k * P:(k + 1) * P],
                                 ones_row[:, 0:1])
            nc.vector.tensor_copy(s_colf[:], scol_ps[:])

        # ---------- Q/K/V projections (fp8 DoubleRow) + AllGather ----------
        with tc.tile_pool(name="qkvps", bufs=4, space="PSUM") as qkv_ps, \
             tc.tile_pool(name="kvst", bufs=4) as kvst:

            # K first: it gates the AllGather. Own K/V go to DRAM only;
            # k_sb / v4 are filled from the gathered buffer (SPMD-uniform).
            for co in range(NCC):
                ps = qkv_ps.tile([P, TQ], f32, tag="ps", name="kps")
                for g in range(NPP):
                    nc.tensor.matmul(
                        ps[:],
                        qkv83[g][:, :,
                                 D_MODEL + co * P:D_MODEL + (co + 1) * P],
                        x83[g][:, :, :], start=(g == 0), stop=False,
                        perf_mode=DR)
                nc.tensor.matmul(
                    ps[:],
                    wbar_sb[:, D_MODEL + co * P:D_MODEL + (co + 1) * P],
                    nmu_row[:], start=False, stop=True)
                kst = kvst.tile([P, TQ], f8, tag="kvs", name="kst")
                with nc.allow_low_precision(reason="fp8 k"):
                    nc.vector.tensor_mul(kst[:], ps[:], sbb[:])
                nc.sync.dma_start(kv_own[co * P:(co + 1) * P, :], kst[:])

            # V (token-major): out[token, feature]
            kvv = kv_own[D_MODEL:2 * D_MODEL, :].rearrange(
                "(p two) n -> p two n", two=2)
            for tcn in range(GROUPS):
                for vh in range(2):
                    ps = qkv_ps.tile([P, TQ], f32, tag="ps", name="vps")
                    for g in range(NPP):
                        nc.tensor.matmul(
                            ps[:],
                            x83[g][:, :, tcn * P:(tcn + 1) * P],
                            qkv83[g][:, :, 2 * D_MODEL + vh * TQ:
                                     2 * D_MODEL + (vh + 1) * TQ],
                            start=(g == 0), stop=False, perf_mode=DR)
                    nc.tensor.matmul(
                        ps[:], nmu_row[:, tcn * P:(tcn + 1) * P],
                        wbar_sb[:, 2 * D_MODEL + vh * TQ:
                                2 * D_MODEL + (vh + 1) * TQ],
                        start=False, stop=True)
                    vst = kvst.tile([P, TQ], f8, tag="kvs", name="vst")
                    with nc.allow_low_precision(reason="fp8 v"):
                        nc.vector.tensor_scalar_mul(vst[:], ps[:],
                                                    s_colf[:, tcn:tcn + 1])
                    nc.sync.dma_start(
                        kv_own[D_MODEL + tcn * 2 * P:
                               D_MODEL + (tcn + 1) * 2 * P, :].rearrange(
                                   "(p two) n -> p (two n)", two=2
                               )[:, vh * TQ:(vh + 1) * TQ],
                        vst[:])

            nc.gpsimd.collective_compute(
                "AllGather",
                mybir.AluOpType.bypass,
                replica_groups=[[0, 1, 2, 3], [4, 5, 6, 7]],
                ins=[kv_own[:]],
                outs=[kv_all[:]],
            )

            # Q projection overlaps the collective
            for co in range(NCC):
                ps = qkv_ps.tile([P, TQ], f32, tag="ps", name="qps")
                for g in range(NPP):
                    nc.tensor.matmul(
                        ps[:], qkv83[g][:, :, co * P:(co + 1) * P],
                        x83[g][:, :, :], start=(g == 0), stop=False,
                        perf_mode=DR)
                nc.tensor.matmul(ps[:], wbar_sb[:, co * P:(co + 1) * P],
                                 nmu_row[:], start=False, stop=True)
                qt = kvst.tile([P, TQ], bf16, tag="qt", name="qt")
                nc.vector.tensor_mul(qt[:], ps[:], sbb[:])
                nc.scalar.activation(q_sb[co][:], qt[:], AF.Identity,
                                     bias=bcol(QB, co))

            # gathered K: one DMA per feature chunk (4 ranks incl. own)
            kall3 = kv_all[:].rearrange("(r c) n -> c r n", r=GROUPS)
            for co in range(NCC):
                nc.sync.dma_start(
                    k_sb[co].rearrange("p (r n) -> p r n", r=GROUPS),
                    kall3[co * P:(co + 1) * P, :, :])
            # gathered V: one DMA per global key chunk
            for gk in range(NKC):
                r, tcn = gk // GROUPS, gk % GROUPS
                base = r * 2 * D_MODEL + D_MODEL + tcn * 2 * P
                src = kv_all[base:base + 2 * P, :].rearrange(
                    "(p two) n -> p (two n)", two=2).rearrange(
                    "p (h c) -> p h c", c=HEAD_DIM)
                nc.sync.dma_start(v4[gk // 2][:, gk % 2, :, 0:HEAD_DIM], src)

        mid.close()  # x8 / qkv8 arenas die; FFN pools reuse the space

        # FFN arenas open now so the w1 DMA overlaps the attention phase.
        ffn_mid = ExitStack()
        h1a = ffn_mid.enter_context(tc.tile_pool(name="h1a", bufs=1))
        w1_ctx = ExitStack()
        w1a = w1_ctx.enter_context(tc.tile_pool(name="w1a", bufs=1))

        h1 = [h1a.tile([P, TQ], bf16, tag=f"h1_{i}", name=f"h1_{i}")
              for i in range(NFC)]
        # w1 in four 2MB phases, double-buffered so each phase's DMA hides
        # under the previous phase's matmuls (same 32KB/partition footprint)
        w1buf = [[None] * NCC, [None] * NCC]

        def load_w1_phase(p):
            for ci in range(NCC):
                wt = w1a.tile([P, 1024], bf16, tag=f"w1{ci}_{p % 2}",
                              name=f"w1t{ci}p{p}")
                nc.sync.dma_start(
                    wt[:], w1_d[ci * P:(ci + 1) * P,
                                p * 1024:(p + 1) * 1024])
                w1buf[p % 2][ci] = wt

        load_w1_phase(0)

        # O weights prefetched before attention as well
        ow8 = []
        for g in range(NPP):
            wt = ow8a.tile([P, 2 * D_MODEL], f8, tag=f"ow{g}",
                           name=f"ow8_{g}")
            nc.sync.dma_start(wt[:], ow8_d[g * P:(g + 1) * P, :])
            ow8.append(wt)
        ow83 = [t.rearrange("p (two n) -> p two n", two=2) for t in ow8]

        # ---------------- attention ----------------
        with tc.tile_pool(name="scps", bufs=2, space="PSUM") as sc_ps, \
             tc.tile_pool(name="pvps", bufs=2, space="PSUM") as pv_psp, \
             tc.tile_pool(name="rbps", bufs=2, space="PSUM") as rb_psp, \
             tc.tile_pool(name="atmp", bufs=2) as atmp:

            # The rb broadcast matmul for head h depends on the 3.35us DVE
            # reciprocal of its Z row; emitting it directly after PV(h)
            # stalls the PE FIFO (the next head's ready score matmuls sit
            # behind it). Defer each head's normalize to after the NEXT
            # head's scores+PV, by which time the recip has finished.
            pending = []

            def emit_norm(ct, ro, pv, rcp):
                rb = rb_psp.tile([64, TQ], f32, tag="rb", name="rb_ps")
                nc.tensor.matmul(rb[:], ones_row[:, 0:64], rcp[:])
                rbs = atmp.tile([64, TQ], f32, tag="rbs", name="rbs")
                nc.vector.tensor_copy(rbs[:], rb[:])
                with nc.allow_low_precision(reason="fp8 attn"):
                    nc.vector.tensor_mul(
                        attn8[ct // 2][ro:ro + 64,
                                       (ct % 2) * TQ:(ct % 2 + 1) * TQ],
                        pv[0:64, :], rbs[:])

            for hd in range(N_HEAD):
                ct, ro = hd // 2, (hd % 2) * 64
                ksl = k_sb[ct][ro:ro + 64, :]
                qsl = q_sb[ct][ro:ro + 64, :]
                eb = (hd % 2) * NKP
                for tp in range(NKP):
                    sc = sc_ps.tile([P, 2 * TQ], f32, tag="sc", name="sc_ps")
                    nc.tensor.matmul(sc[:, 0:TQ],
                                     ksl[:, (2 * tp) * P:(2 * tp + 1) * P],
                                     qsl)
                    nc.tensor.matmul(sc[:, TQ:2 * TQ],
                                     ksl[:, (2 * tp + 1) * P:
                                         (2 * tp + 2) * P], qsl)
                    with nc.allow_low_precision(reason="fp8 es"):
                        nc.scalar.activation(es_t[eb + tp][:], sc[:], AF.Exp,
                                             scale=rs)
                pv = pv_psp.tile([80, TQ], f32, tag="pv", name="pv_ps")
                for pp in range(NKP):
                    nc.tensor.matmul(pv[:], v4[pp][:, :, hd, :],
                                     es3[eb + pp][:, :, :],
                                     start=(pp == 0), stop=(pp == NKP - 1),
                                     perf_mode=DR)
                rcp = atmp.tile([1, TQ], bf16, tag="rc", name="recip")
                with nc.allow_low_precision(reason="bf16 recip bcast"):
                    nc.vector.reciprocal(rcp[:], pv[64:65, :])
                pending.append((ct, ro, pv, rcp))
                if len(pending) > 1:
                    emit_norm(*pending.pop(0))
            emit_norm(*pending.pop(0))

        # ---------------- O projection (fp8 DoubleRow) ----------------
        load_w1_phase(1)  # overlaps O-proj + LN2
        with tc.tile_pool(name="ops", bufs=2, space="PSUM") as o_ps:
            for co in range(NCC):
                ps = o_ps.tile([P, TQ], f32, tag="ps", name="o_ps")
                for g in range(NPP):
                    nc.tensor.matmul(ps[:],
                                     ow83[g][:, :, co * P:(co + 1) * P],
                                     attn83[g][:, :, :],
                                     start=(g == 0), stop=(g == NPP - 1),
                                     perf_mode=DR)
                # x2 = ps/KO + (x_own + ob_eff)   (o bias pre-added to xo)
                nc.vector.scalar_tensor_tensor(x2[co][:], ps[:], bcol(DSC),
                                               xo[co][:], OP.mult, OP.add)
                nc.vector.tensor_copy(xbc[co][:], x2[co][:])

        # ---------------- LN2 ----------------
        with tc.tile_pool(name="xq2", bufs=2) as xqp, \
             tc.tile_pool(name="ln2ps", bufs=1, space="PSUM") as ln2ps, \
             tc.tile_pool(name="ln2bc", bufs=1, space="PSUM") as ln2bc, \
             tc.tile_pool(name="ln2t", bufs=1) as ln2t:
            st2 = ln2ps.tile([33, TQ], f32, tag="st2", name="st2")
            for ci in range(NCC):
                xq = xqp.tile([P, TQ], bf16, tag="xq", name="xq2")
                nc.vector.tensor_mul(xq[:], xbc[ci][:], xbc[ci][:])
                nc.tensor.matmul(st2[0:1, :], ones_bf[:], xbc[ci][:],
                                 start=(ci == 0), stop=(ci == NCC - 1))
                nc.tensor.matmul(st2[32:33, :], ones_bf[:], xq[:],
                                 start=(ci == 0), stop=(ci == NCC - 1))
            mu2 = ln2t.tile([1, TQ], f32, tag="mu2", name="mu2")
            nc.scalar.mul(mu2[:], st2[0:1, :], inv_n)
            musq2 = ln2t.tile([1, TQ], f32, tag="musq2", name="musq2")
            nc.scalar.square(musq2[:], mu2[:])
            vpe2 = ln2t.tile([1, TQ], f32, tag="vpe2", name="vpe2")
            nc.vector.tensor_scalar(vpe2[:], st2[32:33, :], inv_n, 1e-5,
                                    OP.mult, OP.add)
            nc.vector.tensor_sub(vpe2[:], vpe2[:], musq2[:])
            rvar2 = ln2t.tile([1, TQ], f32, tag="rvar2", name="rvar2")
            nc.vector.reciprocal_approx_fast(rvar2[:], vpe2[:])
            s2_row = ln2t.tile([1, TQ], bf16, tag="s2row", name="s2_row")
            with nc.allow_low_precision(reason="bf16 LN2 rows"):
                nc.scalar.activation(s2_row[:], rvar2[:], AF.Sqrt)
                nc.scalar.mul(nmu2_row[:], mu2[:], -1.0)
            sb2_ps = ln2bc.tile([P, TQ], f32, tag="sb2", name="sb2_ps")
            nc.tensor.matmul(sb2_ps[:], ones_row[:], s2_row[:])
            nc.scalar.copy(s2bb[:], sb2_ps[:])

        # ---------------- FFN1 (bf16) ----------------
        with tc.tile_pool(name="h1ps", bufs=4, space="PSUM") as h1_ps, \
             tc.tile_pool(name="drt", bufs=4) as drt_pool:
            for fp in range(4):
                w1t = list(w1buf[fp % 2])
                for fo in range(8):
                    fch = fp * 8 + fo
                    ps = h1_ps.tile([P, TQ], f32, tag="ps", name="h1_ps")
                    for ci in range(NCC):
                        nc.tensor.matmul(ps[:],
                                         w1t[ci][:, fo * P:(fo + 1) * P],
                                         xbc[ci][:], start=(ci == 0),
                                         stop=False)
                    nc.tensor.matmul(ps[:],
                                     wbar1_sb[:, fch * P:(fch + 1) * P],
                                     nmu2_row[:], start=False, stop=True)
                    drt = drt_pool.tile([P, TQ], bf16, tag="drt", name="drt")
                    nc.vector.tensor_mul(drt[:], ps[:], s2bb[:])
                    nc.scalar.activation(h1[fch][:], drt[:], AF.Gelu,
                                         bias=bcol(B1, fch))
                if fp + 2 < 4:
                    load_w1_phase(fp + 2)
        w1_ctx.close()

        # ---------------- FFN2 (bf16) ----------------
        with tc.tile_pool(name="w2rot", bufs=1) as w2rot, \
             tc.tile_pool(name="outps", bufs=1, space="PSUM") as out_ps, \
             tc.tile_pool(name="outsb", bufs=1) as out_pool:
            ops = [out_ps.tile([P, TQ], f32, tag=f"o{co}", name=f"out_ps{co}")
                   for co in range(NCC)]
            for fch in range(NFC):
                wt = w2rot.tile([P, D_MODEL], bf16, tag=f"w2{fch % 8}",
                                name=f"w2t{fch}")
                nc.sync.dma_start(wt[:], w2_d[fch * P:(fch + 1) * P, :])
                for co in range(NCC):
                    nc.tensor.matmul(ops[co][:], wt[:, co * P:(co + 1) * P],
                                     h1[fch][:],
                                     start=(fch == 0), stop=(fch == NFC - 1))
            for co in range(NCC):
                osb = out_pool.tile([P, TQ], f32, tag=f"os{co}",
                                    name=f"osb{co}")
                nc.vector.scalar_tensor_tensor(osb[:], ops[co][:],
                                               bcol(B2, co), x2[co][:],
                                               OP.add, OP.add)
                nc.sync.dma_start(out_d[co * P:(co + 1) * P, :], osb[:])

        if dbg is not None:
            nc.sync.dma_start(dbg["dbg_k0"], k_sb[0][:])
            nc.sync.dma_start(dbg["dbg_v0"], v_sb[0][:])
            nc.sync.dma_start(dbg["dbg_q0"], q_sb[0][:])
            nc.sync.dma_start(dbg["dbg_es0"], es_t[0][:])
            nc.sync.dma_start(dbg["dbg_x20"], x2[0][:])
            nc.sync.dma_start(dbg["dbg_sbb"], sbb[:])
            nc.sync.dma_start(dbg["dbg_nmu"], nmu_row[:])
            nc.sync.dma_start(dbg["dbg_kvall"], kv_all[:])
            nc.sync.dma_start(dbg["dbg_attn0"], attn8[0][:])

        ffn_mid.close()


def _prep_inputs(x, qkv_w, qkv_b, o_w, o_b, ln1_g, ln1_b,
                 ffn_w1, ffn_b1, ffn_w2, ffn_b2, ln2_g, ln2_b):
    import ml_dtypes
    bf = ml_dtypes.bfloat16
    f8 = ml_dtypes.float8_e4m3fn
    fd = np.float64

    def to_f8(a, scale=1.0):
        return np.clip(a * scale, -240.0, 240.0).astype(np.float32).astype(f8)

    def pack_pairs(a):
        # [K, M] -> [K/2, 2*M] DoubleRow pair layout:
        # out[g*128+p, i*M+m] = a[g*256+i*128+p, m]
        K, M = a.shape
        return np.ascontiguousarray(
            a.reshape(K // 256, 2, P, M).transpose(0, 2, 1, 3).reshape(
                K // 2, 2 * M))

    # fold LN1 gamma into qkv weights; beta+bias into constants
    Wg = qkv_w.astype(fd) * ln1_g.astype(fd)[None, :]
    cvec = qkv_w.astype(fd) @ ln1_b.astype(fd) + qkv_b.astype(fd)
    qkv_wT = np.ascontiguousarray(Wg.T.astype(np.float32))  # [C, 3C]
    qkv8p = pack_pairs(to_f8(qkv_wT, KQ_SCALE))
    wbar = np.ascontiguousarray(
        (Wg.sum(axis=1) * KQ_SCALE).astype(np.float32)[None, :]).astype(bf)
    ob_eff = (o_b.astype(fd) + o_w.astype(fd) @ cvec[2 * D_MODEL:]
              ).astype(np.float32)

    o_wT = np.ascontiguousarray(o_w.T.astype(np.float32))   # [C, C]
    ow8p = pack_pairs(to_f8(o_wT, KO_SCALE))

    W1g = ffn_w1.astype(fd) * ln2_g.astype(fd)[None, :]
    b1_eff = (ffn_w1.astype(fd) @ ln2_b.astype(fd)
              + ffn_b1.astype(fd)).astype(np.float32)
    w1T = np.ascontiguousarray(W1g.T.astype(np.float32)).astype(bf)
    wbar1 = np.ascontiguousarray(
        W1g.sum(axis=1).astype(np.float32)[None, :]).astype(bf)
    w2T = np.ascontiguousarray(ffn_w2.T).astype(bf)

    def cols(v, n):
        return np.ascontiguousarray(v.reshape(n, P).T.astype(np.float32))

    biases = np.zeros((P, 64), np.float32)
    biases[:, QB:QB + 8] = cols(cvec[0:D_MODEL].astype(np.float32), 8)
    biases[:, B1:B1 + 32] = cols(b1_eff, 32)
    biases[:, B2:B2 + 8] = cols(ffn_b2, 8)
    biases[:, DSC] = 1.0 / KO_SCALE

    in_maps = []
    for c in range(N_CORES):
        b, s = c // GROUPS, c % GROUPS
        xT = x[b].T  # [C, T] feature-major
        x_own = np.ascontiguousarray(xT[:, s * TQ:(s + 1) * TQ])
        x8p = pack_pairs(to_f8(x_own.astype(np.float32)))
        in_maps.append({
            "x8p": x8p,
            "x_own": np.ascontiguousarray(
                x_own.astype(np.float32) + ob_eff[:, None]),
            "qkv8p": qkv8p,
            "ow8p": ow8p,
            "wbar": wbar,
            "wbar1": wbar1,
            "w1T": w1T,
            "w2T": w2T,
            "biases": biases,
        })
    return in_maps


def kernel(**inputs):
    from concourse.bass_utils import run_bass_kernel_spmd

    if "nc" not in _cache:
        _cache["nc"] = _build()
    nc = _cache["nc"]

    inputs = {k: np.asarray(v, dtype=np.float32) for k, v in inputs.items()}
    in_maps = _prep_inputs(**inputs)

    res = run_bass_kernel_spmd(nc, in_maps, core_ids=list(range(N_CORES)),
                               **_cache.get("run_kwargs", {}))
    _cache["last_results"] = res

    out = np.empty((B, T, D_MODEL), np.float32)
    for c in range(N_CORES):
        b, s = c // GROUPS, c % GROUPS
        out[b, s * TQ:(s + 1) * TQ, :] = res.results[c]["out"].T
    return out
